# revision 1
# baseline (speedup 1.0000x reference)
"""PINN loss kernel for trn2 (8 NeuronCores, data-parallel over points).

Computes the 9 scalar losses of the PhysicsInformedNN reference:
MLP forward + first/second x,y-derivatives via fused forward-mode AD
streams (h, t_x, t_y, s_x, s_y) propagated through the 8-128-128-128-128-4
tanh MLP, then RANS/BC/inlet/incompressibility MSE losses.

Layout: features on partitions, points on the free dim; per-core shard of
6250 points processed in 13 tiles of <=512 points. Per-core partial sums
[128, 8] are combined on host (the unshard step).
"""

import os
import sys

for _p in ("/opt/trn_rl_repo", "/root/.axon_site/_ro/trn_rl_repo"):
    if os.path.isdir(_p) and _p not in sys.path:
        sys.path.insert(0, _p)

import numpy as np
from contextlib import ExitStack

from concourse import bass, bacc, tile, mybir
from concourse.bass_utils import run_bass_kernel_spmd

NCORES = 8
NPTS = 50000
NPC = NPTS // NCORES          # 6250 points per core
FD = 512                      # points per tile
NT = (NPC + FD - 1) // FD     # 13 tiles (12 full + one of 106)
NCH = (NPC + 127) // 128      # 49 transpose chunks per core
NU = 1.56e-05
EPS = 1e-08

F32 = mybir.dt.float32
F32R = mybir.dt.float32r
AF = mybir.ActivationFunctionType
ALU = mybir.AluOpType

MMD = F32R if os.environ.get("PINN_MM_DTYPE", "f32r") == "f32r" else F32
DBG_NT = int(os.environ.get("PINN_NT", NT))          # tiles to emit (debug)
DBG_L4 = os.environ.get("PINN_SKIP_L4", "") == ""    # emit L4+transpose
DBG_PW = os.environ.get("PINN_SKIP_PW", "") == ""    # emit pointwise phase
DBG_STAGE = int(os.environ.get("PINN_PW_STAGE", "9"))

_CACHE = {}


def _build():
    nc = bacc.Bacc("TRN2", target_bir_lowering=False, debug=False)

    # ---- DRAM I/O ----
    d_feat = nc.dram_tensor("feat8", [8, NPC], F32, kind="ExternalInput")
    d_w0 = nc.dram_tensor("w0", [8, 128], F32, kind="ExternalInput")
    d_w1 = nc.dram_tensor("w1", [128, 128], F32, kind="ExternalInput")
    d_w2 = nc.dram_tensor("w2", [128, 128], F32, kind="ExternalInput")
    d_w3 = nc.dram_tensor("w3", [128, 128], F32, kind="ExternalInput")
    d_w3n = nc.dram_tensor("w3n", [128, 128], F32, kind="ExternalInput")
    d_w4sp = nc.dram_tensor("w4sp", [128, 100], F32, kind="ExternalInput")
    d_bias = nc.dram_tensor("bias", [128, 4], F32, kind="ExternalInput")
    d_b4s = nc.dram_tensor("b4s", [20, 1], F32, kind="ExternalInput")
    d_winit = nc.dram_tensor("winit", [128, 4], F32, kind="ExternalInput")
    d_cnb = nc.dram_tensor("cnb", [128, 8], F32, kind="ExternalInput")
    d_tins = nc.dram_tensor("tins", [128, 6 * NCH], F32, kind="ExternalInput")
    d_mask = nc.dram_tensor("mask", [128, NCH], F32, kind="ExternalInput")
    d_id = nc.dram_tensor("ident", [128, 128], F32, kind="ExternalInput")
    d_out = nc.dram_tensor("sout", [128, 8], F32, kind="ExternalOutput")

    with tile.TileContext(nc) as tc, ExitStack() as ctx:
        wp = ctx.enter_context(tc.tile_pool(name="wp", bufs=1))
        sb = ctx.enter_context(tc.tile_pool(name="sb", bufs=int(os.environ.get("PINN_SBUFS","3"))))
        scr = ctx.enter_context(tc.tile_pool(name="scr", bufs=26))
        pz = ctx.enter_context(tc.tile_pool(name="pz", bufs=int(os.environ.get("PINN_PZBUFS","2")), space="PSUM"))
        pt = ctx.enter_context(tc.tile_pool(name="pt", bufs=2, space="PSUM"))
        ps = ctx.enter_context(tc.tile_pool(name="ps", bufs=1, space="PSUM"))
        

        # ---- persistent sbuf tensors ----
        feat0 = wp.tile([8, NPC], F32, tag="feat0")
        feat = wp.tile([8, NPC], MMD, tag="feat")
        w0 = wp.tile([8, 128], MMD, tag="w0")
        w00 = wp.tile([8, 128], F32, tag="w00")
        w0123 = wp.tile([128, 128 * 4], F32, tag="w0123")
        w4sp0 = wp.tile([128, 100], F32, tag="w4sp0")
        w1 = wp.tile([128, 128], MMD, tag="w1")
        w2 = wp.tile([128, 128], MMD, tag="w2")
        w3 = wp.tile([128, 128], MMD, tag="w3")
        w3n = wp.tile([128, 128], MMD, tag="w3n")
        w4sp = wp.tile([128, 100], MMD, tag="w4sp")
        bias = wp.tile([128, 4], F32, tag="bias")
        b4s = wp.tile([20, 1], F32, tag="b4s")
        winit = wp.tile([128, 4], F32, tag="winit")
        cnb = wp.tile([128, 8], F32, tag="cnb")
        tins = wp.tile([128, 6 * NCH], F32, tag="tins")
        mask = wp.tile([128, NCH], F32, tag="mask")
        ident = wp.tile([128, 128], F32, tag="ident")
        qall = wp.tile([128, 20 * NCH], F32, tag="qall")
        souts = wp.tile([128, 8], F32, tag="souts")

        warm = wp.tile([1, 8], F32, tag="warm")
        nc.gpsimd.memset(warm[:, :], 0.25)
        nc.scalar.activation(warm[:, :], warm[:, :], AF.Tanh)
        nc.scalar.activation(warm[:, :], warm[:, :], AF.Square)
        dma = nc.sync.dma_start
        dma(out=feat0[:, :], in_=d_feat[:, :])
        dma(out=w00[:, :], in_=d_w0[:, :])
        dma(out=w0123[:, 0:128], in_=d_w1[:, :])
        dma(out=w0123[:, 128:256], in_=d_w2[:, :])
        dma(out=w0123[:, 256:384], in_=d_w3[:, :])
        dma(out=w0123[:, 384:512], in_=d_w3n[:, :])
        dma(out=w4sp0[:, :], in_=d_w4sp[:, :])
        nc.vector.tensor_copy(feat[:, 0:FD], feat0[:, 0:FD])
        nc.vector.tensor_copy(feat[:, FD:], feat0[:, FD:])
        nc.vector.tensor_copy(w0[:, :], w00[:, :])
        nc.vector.tensor_copy(w1[:, :], w0123[:, 0:128])
        nc.vector.tensor_copy(w2[:, :], w0123[:, 128:256])
        nc.vector.tensor_copy(w3[:, :], w0123[:, 256:384])
        nc.vector.tensor_copy(w3n[:, :], w0123[:, 384:512])
        nc.vector.tensor_copy(w4sp[:, :], w4sp0[:, :])
        dma(out=bias[:, :], in_=d_bias[:, :])
        dma(out=b4s[:, :], in_=d_b4s[:, :])
        dma(out=winit[:, :], in_=d_winit[:, :])
        dma(out=cnb[:, :], in_=d_cnb[:, :])
        dma(out=tins[:, :], in_=d_tins[:, :])
        dma(out=mask[:, :], in_=d_mask[:, :])
        dma(out=ident[:, :], in_=d_id[:, :])
        nc.gpsimd.memset(qall[:, :], 0.0)

        hidW = [w1, w2, w3]
        csgn = [2.0, -2.0, 2.0]  # c_l for layers l=2,3,4

        def emit_head(t):
            off = t * FD
            fd = min(FD, NPC - off)

            # ---- L0 matmul: z1 = W0^T feat ----
            zh = pz.tile([128, fd], F32, tag="zh", name=f"zh0_{t}")
            nc.tensor.matmul(zh[:, :], (w0[:, :]), (feat[:, off:off + fd]),
                             start=True, stop=True)

            # ---- EL1 ----
            th = sb.tile([128, fd], MMD, tag="th", name=f"th1_{t}")
            nc.scalar.activation(th[:, :], zh[:, :], AF.Tanh, bias=bias[:, 0:1])
            q = sb.tile([128, fd], F32, tag="q", name=f"q1_{t}")
            nc.scalar.activation(q[:, :], th[:, :], AF.Square)
            qm1 = sb.tile([128, fd], F32, tag="qm1", name=f"qm11_{t}")
            nc.vector.tensor_scalar_add(qm1[:, :], q[:, :], -1.0)
            thgm = sb.tile([128, fd], F32, tag="thgm", name=f"thgm1_{t}")
            nc.vector.tensor_tensor(thgm[:, :], qm1[:, :], th[:, :], ALU.mult)
            t2o = sb.tile([128, 2 * fd], MMD, tag="t2o", name=f"t2o1_{t}")
            s2o = sb.tile([128, 2 * fd], MMD, tag="s2o", name=f"s2o1_{t}")
            # stored T_1d = (Q-1)*w0d ; S_1d = THGM * (-2*w0d^2)
            nc.vector.tensor_scalar_mul(t2o[:, 0:fd], qm1[:, :], winit[:, 0:1])
            nc.vector.tensor_scalar_mul(t2o[:, fd:2 * fd], qm1[:, :], winit[:, 1:2])
            nc.vector.tensor_scalar_mul(s2o[:, 0:fd], thgm[:, :], winit[:, 2:3])
            nc.vector.tensor_scalar_mul(s2o[:, fd:2 * fd], thgm[:, :], winit[:, 3:4])
            pp = None

            # ---- hidden layers l = 2,3,4 ----
            for li in range(3):
                W = hidW[li]
                zh = pz.tile([128, fd], F32, tag="zh", name=f"zh{li}_{t}")
                zt2 = pt.tile([128, 2 * fd], F32, tag="zt2", name=f"zt{li}_{t}")
                zs2 = ps.tile([128, 2 * fd], F32, tag="zs2", name=f"zs{li}_{t}")
                nc.tensor.matmul(zh[:, :], (W[:, :]), (th[:, :]),
                                 start=True, stop=True)
                nc.tensor.matmul(zt2[:, 0:fd], (W[:, :]), (t2o[:, 0:fd]),
                                 start=True, stop=True)
                nc.tensor.matmul(zt2[:, fd:2 * fd], (W[:, :]), (t2o[:, fd:2 * fd]),
                                 start=True, stop=True)
                if pp is None:
                    nc.tensor.matmul(zs2[:, 0:fd], (W[:, :]), (s2o[:, 0:fd]),
                                     start=True, stop=True)
                    nc.tensor.matmul(zs2[:, fd:2 * fd], (W[:, :]),
                                     (s2o[:, fd:2 * fd]), start=True, stop=True)
                else:
                    Wp = w3n if li == 2 else W
                    nc.tensor.matmul(zs2[:, 0:fd], (W[:, :]), (s2o[:, 0:fd]),
                                     start=True, stop=False)
                    nc.tensor.matmul(zs2[:, 0:fd], (Wp[:, :]), (pp[:, 0:fd]),
                                     start=False, stop=True)
                    nc.tensor.matmul(zs2[:, fd:2 * fd], (W[:, :]),
                                     (s2o[:, fd:2 * fd]), start=True, stop=False)
                    nc.tensor.matmul(zs2[:, fd:2 * fd], (Wp[:, :]),
                                     (pp[:, fd:2 * fd]), start=False, stop=True)

                th = sb.tile([128, fd], MMD, tag="th", name=f"th{li}_{t}")
                nc.scalar.activation(th[:, :], zh[:, :], AF.Tanh,
                                     bias=bias[:, li + 1:li + 2])
                q = sb.tile([128, fd], F32, tag="q", name=f"q{li}_{t}")
                nc.scalar.activation(q[:, :], th[:, :], AF.Square)
                qm1 = sb.tile([128, fd], F32, tag="qm1", name=f"qm1{li}_{t}")
                nc.vector.tensor_scalar_add(qm1[:, :], q[:, :], -1.0)
                thgm = sb.tile([128, fd], F32, tag="thgm", name=f"thgm{li}_{t}")
                nc.vector.tensor_tensor(thgm[:, :], qm1[:, :], th[:, :], ALU.mult)
                # zx2s = 2 * zt^2 via Square(sqrt(2) * zt)
                zx2 = sb.tile([128, 2 * fd], F32, tag="zx2", name=f"zx2{li}_{t}")
                nc.scalar.activation(zx2[:, :], zt2[:, :], AF.Square,
                                     scale=1.4142135623730951)
                # stored T/SA = (Q-1) o Z
                t2o = sb.tile([128, 2 * fd], MMD, tag="t2o", name=f"t2o{li}_{t}")
                s2o = sb.tile([128, 2 * fd], MMD, tag="s2o", name=f"s2o{li}_{t}")
                qb = q[:, :].unsqueeze(1).broadcast_to([128, 2, fd])
                nc.vector.scalar_tensor_tensor(
                    t2o[:, :].rearrange("p (r f) -> p r f", r=2), qb, 1.0,
                    zt2[:, :].rearrange("p (r f) -> p r f", r=2),
                    ALU.subtract, ALU.mult)
                nc.vector.scalar_tensor_tensor(
                    s2o[:, :].rearrange("p (r f) -> p r f", r=2), qb, 1.0,
                    zs2[:, :].rearrange("p (r f) -> p r f", r=2),
                    ALU.subtract, ALU.mult)
                pp = sb.tile([128, 2 * fd], MMD, tag="pp", name=f"pp{li}_{t}")
                nc.gpsimd.tensor_tensor(pp[:, 0:fd], zx2[:, 0:fd], thgm[:, :],
                                        ALU.mult)
                nc.gpsimd.tensor_tensor(pp[:, fd:2 * fd], zx2[:, fd:2 * fd],
                                        thgm[:, :], ALU.mult)
            return th, t2o, s2o, pp

        def emit_tail(t, th, t2o, s2o, pp):
            if not DBG_L4:
                return
            off = t * FD
            fd = min(FD, NPC - off)
            nchunks = (fd + 127) // 128

            # ---- L4: stacked output matmuls into o20 [20, fd] ----
            o20 = pz.tile([20, fd], F32, tag="zh", name=f"o20_{t}")
            nc.tensor.matmul(o20[:, :], (w4sp[:, 0:20]), (th[:, :]),
                             start=True, stop=False)
            nc.tensor.matmul(o20[:, :], (w4sp[:, 20:40]), (t2o[:, 0:fd]),
                             start=False, stop=False)
            nc.tensor.matmul(o20[:, :], (w4sp[:, 40:60]), (t2o[:, fd:2 * fd]),
                             start=False, stop=False)
            nc.tensor.matmul(o20[:, :], (w4sp[:, 60:80]), (s2o[:, 0:fd]),
                             start=False, stop=False)
            nc.tensor.matmul(o20[:, :], (w4sp[:, 60:80]), (pp[:, 0:fd]),
                             start=False, stop=False)
            nc.tensor.matmul(o20[:, :], (w4sp[:, 80:100]), (s2o[:, fd:2 * fd]),
                             start=False, stop=False)
            nc.tensor.matmul(o20[:, :], (w4sp[:, 80:100]), (pp[:, fd:2 * fd]),
                             start=False, stop=True)

            o20sb = sb.tile([20, fd], F32, tag="o20sb", name=f"o20sb_{t}")
            nc.scalar.activation(o20sb[:, :], o20[:, :], AF.Identity,
                                 bias=b4s[:, 0:1])

            # ---- transpose to points-on-partitions ----
            qt = pz.tile([128, 20 * nchunks], F32, tag="zh", name=f"qt_{t}")
            for ci in range(nchunks):
                w = min(128, fd - ci * 128)
                nc.tensor.transpose(qt[0:w, ci * 20:(ci + 1) * 20],
                                    o20sb[:, ci * 128:ci * 128 + w],
                                    ident[0:20, 0:20])
            gw = min(128, fd - (nchunks - 1) * 128)
            if gw == 128:
                nc.scalar.copy(qall[:, t * 80:t * 80 + 20 * nchunks], qt[:, :])
            else:
                if nchunks > 1:
                    nc.scalar.copy(qall[:, t * 80:t * 80 + 20 * (nchunks - 1)],
                                   qt[:, 0:20 * (nchunks - 1)])
                nc.scalar.copy(
                    qall[0:gw, t * 80 + 20 * (nchunks - 1):t * 80 + 20 * nchunks],
                    qt[0:gw, 20 * (nchunks - 1):20 * nchunks])

        prev = None
        for t in range(DBG_NT):
            cur = emit_head(t)
            if prev is not None:
                emit_tail(t - 1, *prev)
            prev = cur
        emit_tail(DBG_NT - 1, *prev)

        # ---- pointwise loss phase on [128, NCH] views ----
        def _pointwise():
            def qv(j):
                return qall[:, j:20 * NCH:20]

            _ctr = [0]

            def new():
                _ctr[0] += 1
                return scr.tile([128, NCH], F32, tag="scr", name=f"scr{_ctr[0]}")

            def tt(a, b, op, eng=nc.vector):
                o = new()
                eng.tensor_tensor(o[:, :], a, b, op)
                return o

            V = nc.vector
            G = nc.vector if os.environ.get("PINN_PW_NOPOOL", "") else nc.gpsimd
            if DBG_STAGE < 1:
                return
            A = tt(qv(4), qv(8), ALU.add, G)            # ux+uy
            B = tt(qv(5), qv(9), ALU.add, V)            # vx+vy
            uv1 = tt(qv(1), A[:, :], ALU.mult, G)       # v*(ux+uy)
            uv2 = tt(qv(0), B[:, :], ALU.mult, V)       # u*(vx+vy)
            uvxy = tt(uv1[:, :], uv2[:, :], ALU.add, V)

            def stt_nu(zz, eng):
                o = new()
                eng.scalar_tensor_tensor(o[:, :], qv(3), NU, zz, ALU.add, ALU.mult)
                return o

            if DBG_STAGE < 2:
                return
            t1 = stt_nu(qv(12), V)   # (nut+NU)*uxx
            t2 = stt_nu(qv(16), V)   # (nut+NU)*uyy
            t3 = tt(qv(7), qv(4), ALU.mult, V)   # nux*ux
            t4 = tt(qv(11), qv(8), ALU.mult, G)  # nuy*uy
            a1 = tt(uvxy[:, :], qv(6), ALU.add, V)
            a2 = tt(t1[:, :], t3[:, :], ALU.add, V)
            a3 = tt(t2[:, :], t4[:, :], ALU.add, G)
            a4 = tt(a2[:, :], a3[:, :], ALU.add, V)
            f_u = tt(a1[:, :], a4[:, :], ALU.subtract, V)

            if DBG_STAGE < 3:
                return
            t5 = stt_nu(qv(13), V)   # (nut+NU)*vxx
            t6 = stt_nu(qv(17), V)   # (nut+NU)*vyy
            t7 = tt(qv(7), qv(5), ALU.mult, V)   # nux*vx
            t8 = tt(qv(11), qv(9), ALU.mult, G)  # nuy*vy
            b1 = tt(uvxy[:, :], qv(10), ALU.add, V)
            b2 = tt(t5[:, :], t7[:, :], ALU.add, V)
            b3 = tt(t6[:, :], t8[:, :], ALU.add, G)
            b4 = tt(b2[:, :], b3[:, :], ALU.add, V)
            f_v = tt(b1[:, :], b4[:, :], ALU.subtract, V)

            if DBG_STAGE < 4:
                return
            ic = tt(qv(4), qv(9), ALU.add, G)

            t1b = new()
            V.tensor_scalar(t1b[:, :], qv(0), cnb[:, 0:1], cnb[:, 1:2], ALU.mult, ALU.add)
            xnb = new()
            V.tensor_scalar(xnb[:, :], tins[:, 0:NCH], cnb[:, 2:3], cnb[:, 3:4],
                            ALU.mult, ALU.add)
            t2b = new()
            V.tensor_scalar(t2b[:, :], qv(1), cnb[:, 4:5], cnb[:, 5:6], ALU.mult, ALU.add)
            ynb = new()
            V.tensor_scalar(ynb[:, :], tins[:, NCH:2 * NCH], cnb[:, 6:7], cnb[:, 7:8],
                            ALU.mult, ALU.add)
            m1 = tt(t1b[:, :], xnb[:, :], ALU.mult, V)
            m2 = tt(t2b[:, :], ynb[:, :], ALU.mult, G)
            bc0 = tt(m1[:, :], m2[:, :], ALU.add, V)
            bc = tt(bc0[:, :], mask[:, :], ALU.mult, V)

            if DBG_STAGE < 5:
                return
            du = tt(tins[:, 2 * NCH:3 * NCH], qv(0), ALU.subtract, V)
            dv = tt(tins[:, 3 * NCH:4 * NCH], qv(1), ALU.subtract, G)
            dp = tt(tins[:, 4 * NCH:5 * NCH], qv(2), ALU.subtract, V)
            dnut = tt(tins[:, 5 * NCH:6 * NCH], qv(3), ALU.subtract, G)

            if DBG_STAGE < 6:
                return
            for k, val in enumerate([f_u, f_v, bc, ic, du, dv, dp, dnut]):
                o = new()
                nc.scalar.activation(o[:, :], val[:, :], AF.Square,
                                     accum_out=souts[:, k:k + 1])

            nc.sync.dma_start(out=d_out[:, :], in_=souts[:, :])

        if DBG_PW:
            _pointwise()
        else:
            nc.sync.dma_start(out=d_out[:, :], in_=qall[:, 0:8])

    nc.compile()
    return nc


def _prep_core(inputs, c):
    s = slice(c * NPC, (c + 1) * NPC)
    f32 = np.float32
    col = lambda k: np.asarray(inputs[k], f32)[s, 0]
    feat = np.ascontiguousarray(np.stack([
        col("x"), col("y"), col("x_normal"), col("y_normal"), col("sdf"),
        col("gamma_1"), col("gamma_2"), col("gamma_3")]))
    W = [np.asarray(inputs[f"W{i}"], f32) for i in range(5)]
    b = [np.asarray(inputs[f"b{i}"], f32) for i in range(5)]
    cn = np.asarray(inputs["coef_norm"], f32)

    w4sp = np.zeros((128, 100), f32)
    for slot in range(5):
        w4sp[:, slot * 20 + slot * 4: slot * 20 + slot * 4 + 4] = W[4]
    bias = np.stack([b[0], b[1], b[2], b[3]], axis=1)
    b4s = np.concatenate([b[4], np.zeros(16, f32)])[:, None]
    w0x, w0y = W[0][0, :], W[0][1, :]
    winit = np.stack([w0x, w0y, -2.0 * w0x * w0x, -2.0 * w0y * w0y], axis=1)
    cnv = np.array([cn[3, 0] + EPS, cn[2, 0], cn[1, 5] + EPS, cn[0, 5],
                    cn[3, 1] + EPS, cn[2, 1], cn[1, 6] + EPS, cn[0, 6]], f32)
    cnb = np.broadcast_to(cnv, (128, 8)).copy()

    def tcol(k):
        a = np.zeros(NCH * 128, f32)
        a[:NPC] = col(k)
        return a.reshape(NCH, 128).T  # [128, NCH]

    tins = np.ascontiguousarray(np.concatenate(
        [tcol("x_normal"), tcol("y_normal"), tcol("u0"), tcol("v0"),
         tcol("p0"), tcol("nut0")], axis=1))
    m = np.zeros(NCH * 128, f32)
    m[:NPC] = 1.0
    mask = np.ascontiguousarray(m.reshape(NCH, 128).T)

    return {
        "feat8": feat, "w0": np.ascontiguousarray(W[0]),
        "w1": np.ascontiguousarray(W[1]), "w2": np.ascontiguousarray(W[2]),
        "w3": np.ascontiguousarray(W[3]), "w3n": np.ascontiguousarray(-W[3]),
        "w4sp": w4sp,
        "bias": np.ascontiguousarray(bias), "b4s": b4s, "winit": np.ascontiguousarray(winit),
        "cnb": cnb, "tins": tins, "mask": mask,
        "ident": np.eye(128, dtype=f32),
    }


def _get_nc():
    if "nc" not in _CACHE:
        _CACHE["nc"] = _build()
    return _CACHE["nc"]


def run_device(inputs, **kw):
    nc = _get_nc()
    in_maps = [_prep_core(inputs, c) for c in range(NCORES)]
    res = run_bass_kernel_spmd(nc, in_maps, core_ids=list(range(NCORES)), **kw)
    return res


def _combine(results):
    S = np.stack([r["sout"] for r in results]).astype(np.float64)  # [8,128,8]
    m = S.sum(axis=(0, 1)) / NPTS
    rans, bcl, icl = m[0] + m[1], m[2], m[3]
    ul, vl, pl, nl = m[4], m[5], m[6], m[7]
    inlet = ul + vl + pl + nl
    total = rans + bcl + inlet + icl
    return np.array([total, rans, bcl, inlet, icl, ul, vl, pl, nl],
                    dtype=np.float32)


def kernel(**inputs):
    res = run_device(inputs)
    return _combine(res.results)



# revision 7
# speedup vs baseline: 1.0463x; 1.0463x over previous
"""PINN loss kernel for trn2 (8 NeuronCores, data-parallel over points).

v2: forward-mode AD with a single Laplacian second-derivative stream
(s = s_xx + s_yy; the losses only need u_xx+u_yy / v_xx+v_yy), bf16
streams + matmuls, layer-1 tangent scales folded into pre-scaled weight
copies (host-side), engine-balanced elementwise work.

Layout: features on partitions, points on the free dim; per-core shard of
6250 points processed in 13 tiles of <=512 points. Per-core partial sums
[128, 8] are combined on host (the unshard step).
"""

import os
import sys

for _p in ("/opt/trn_rl_repo", "/root/.axon_site/_ro/trn_rl_repo"):
    if os.path.isdir(_p) and _p not in sys.path:
        sys.path.insert(0, _p)

import numpy as np
import ml_dtypes
from contextlib import ExitStack

from concourse import bass, bacc, tile, mybir
from concourse.bass_utils import run_bass_kernel_spmd

NCORES = 8
NPTS = 50000
NPC = NPTS // NCORES          # 6250 points per core
FD = 512                      # points per tile
NT = (NPC + FD - 1) // FD     # 13 tiles (12 full + one of 106)
NCH = (NPC + 127) // 128      # 49 transpose chunks per core
NS = 14                       # output slots per point
NU = 1.56e-05
EPS = 1e-08
SQRT2 = 1.4142135623730951

F32 = mybir.dt.float32
BF = mybir.dt.bfloat16
AF = mybir.ActivationFunctionType
ALU = mybir.AluOpType
BF_NP = ml_dtypes.bfloat16

DBG_NT = int(os.environ.get("PINN_NT", NT))          # tiles to emit (debug)
DBG_L4 = os.environ.get("PINN_SKIP_L4", "") == ""    # emit L4+transpose
DBG_PW = os.environ.get("PINN_SKIP_PW", "") == ""    # emit pointwise phase

_CACHE = {}


def _build():
    nc = bacc.Bacc("TRN2", target_bir_lowering=False, debug=False)

    # ---- DRAM I/O ----
    d_feat = nc.dram_tensor("featb", [8, NPC], BF, kind="ExternalInput")
    d_w0 = nc.dram_tensor("w0b", [8, 128], BF, kind="ExternalInput")
    d_wh = nc.dram_tensor("wh", [128, 128 * 3], BF, kind="ExternalInput")
    d_wx = nc.dram_tensor("wx", [128, 128 * 3], BF, kind="ExternalInput")
    d_w2n = nc.dram_tensor("w2n", [128, 128], BF, kind="ExternalInput")
    d_w4 = nc.dram_tensor("w4sp", [128, 5 * NS], BF, kind="ExternalInput")
    d_bias = nc.dram_tensor("bias", [128, 4], F32, kind="ExternalInput")
    d_b4s = nc.dram_tensor("b4s", [NS, 1], F32, kind="ExternalInput")
    d_cnb = nc.dram_tensor("cnb", [128, 8], F32, kind="ExternalInput")
    d_tins = nc.dram_tensor("tins", [128, 6 * NCH], F32, kind="ExternalInput")
    d_mask = nc.dram_tensor("mask", [128, NCH], F32, kind="ExternalInput")
    d_id = nc.dram_tensor("ident", [NS, NS], F32, kind="ExternalInput")
    d_out = nc.dram_tensor("sout", [128, 8], F32, kind="ExternalOutput")

    with tile.TileContext(nc) as tc, ExitStack() as ctx:
        wp = ctx.enter_context(tc.tile_pool(name="wp", bufs=1))
        sb = ctx.enter_context(tc.tile_pool(name="sb", bufs=int(os.environ.get("PINN_SBUFS", "3"))))
        scr = ctx.enter_context(tc.tile_pool(name="scr", bufs=26))
        pa = ctx.enter_context(tc.tile_pool(name="pa", bufs=2, space="PSUM"))
        pb = ctx.enter_context(tc.tile_pool(name="pb", bufs=2, space="PSUM"))

        # ---- persistent sbuf tensors ----
        feat = wp.tile([8, NPC], BF, tag="feat")
        w0 = wp.tile([8, 128], BF, tag="w0")
        wh = wp.tile([128, 128 * 3], BF, tag="wh")
        wx = wp.tile([128, 128 * 3], BF, tag="wx")
        w2n = wp.tile([128, 128], BF, tag="w2n")
        w4 = wp.tile([128, 5 * NS], BF, tag="w4")
        bias = wp.tile([128, 4], F32, tag="bias")
        b4s = wp.tile([NS, 1], F32, tag="b4s")
        cnb = wp.tile([128, 8], F32, tag="cnb")
        tins = wp.tile([128, 6 * NCH], F32, tag="tins")
        mask = wp.tile([128, NCH], F32, tag="mask")
        ident = wp.tile([NS, NS], F32, tag="ident")
        qall = wp.tile([128, NS * NCH], F32, tag="qall")
        souts = wp.tile([128, 8], F32, tag="souts")

        warm = wp.tile([1, 8], F32, tag="warm")
        nc.gpsimd.memset(warm[:, :], 0.25)
        nc.scalar.activation(warm[:, :], warm[:, :], AF.Tanh)
        nc.scalar.activation(warm[:, :], warm[:, :], AF.Square)
        dma = nc.sync.dma_start
        dma(out=feat[:, :], in_=d_feat[:, :])
        dma(out=w0[:, :], in_=d_w0[:, :])
        dma(out=wh[:, :], in_=d_wh[:, :])
        dma(out=wx[:, :], in_=d_wx[:, :])
        dma(out=w2n[:, :], in_=d_w2n[:, :])
        dma(out=w4[:, :], in_=d_w4[:, :])
        dma(out=bias[:, :], in_=d_bias[:, :])
        dma(out=b4s[:, :], in_=d_b4s[:, :])
        dma(out=cnb[:, :], in_=d_cnb[:, :])
        dma(out=tins[:, :], in_=d_tins[:, :])
        dma(out=mask[:, :], in_=d_mask[:, :])
        dma(out=ident[:, :], in_=d_id[:, :])
        nc.gpsimd.memset(qall[:, :], 0.0)

        V, S, G, T = nc.vector, nc.scalar, nc.gpsimd, nc.tensor
        SGN = [-1.0, 1.0, -1.0]  # sign of stored pp term per hidden iter

        def emit_head(t):
            off = t * FD
            fd = min(FD, NPC - off)

            # ---- L0 matmul + EL1 ----
            zh = pa.tile([128, fd], F32, tag="pa", name=f"zh0_{t}")
            T.matmul(zh[:, :], w0[:, :], feat[:, off:off + fd],
                     start=True, stop=True)
            th = sb.tile([128, fd], BF, tag="th", name=f"th0_{t}")
            S.activation(th[:, :], zh[:, :], AF.Tanh, bias=bias[:, 0:1])
            q = sb.tile([128, fd], BF, tag="q", name=f"q0_{t}")
            V.tensor_tensor(q[:, :], th[:, :], th[:, :], ALU.mult)
            qm1 = sb.tile([128, fd], BF, tag="qm1", name=f"qm1_{t}")
            V.tensor_scalar_add(qm1[:, :], q[:, :], -1.0)
            thgm = sb.tile([128, fd], BF, tag="thgm", name=f"thgm0_{t}")
            G.tensor_tensor(thgm[:, :], qm1[:, :], th[:, :], ALU.mult)

            tso = None
            pp = None
            # ---- hidden iterations li=0,1,2 (weights W1,W2,W3) ----
            for li in range(3):
                W = wh[:, li * 128:(li + 1) * 128]
                zh = pa.tile([128, fd], F32, tag="pa", name=f"zh{li}_{t}")
                zz = pb.tile([128, 3 * fd], F32, tag="pb", name=f"zz{li}_{t}")
                if li == 0:
                    T.matmul(zh[:, :], W, th[:, :], start=True, stop=True)
                    T.matmul(zz[:, 0:fd], wx[:, 0:128], qm1[:, :],
                             start=True, stop=True)
                    T.matmul(zz[:, fd:2 * fd], wx[:, 128:256], qm1[:, :],
                             start=True, stop=True)
                    T.matmul(zz[:, 2 * fd:3 * fd], wx[:, 256:384], thgm[:, :],
                             start=True, stop=True)
                else:
                    T.matmul(zh[:, :], W, th[:, :], start=True, stop=True)
                    T.matmul(zz[:, 0:fd], W, tso[:, 0:fd], start=True, stop=True)
                    T.matmul(zz[:, fd:2 * fd], W, tso[:, fd:2 * fd],
                             start=True, stop=True)
                    T.matmul(zz[:, 2 * fd:3 * fd], W, tso[:, 2 * fd:3 * fd],
                             start=True, stop=False)
                    Wpp = w2n[:, :] if li == 1 else W
                    T.matmul(zz[:, 2 * fd:3 * fd], Wpp, pp[:, :],
                             start=False, stop=True)

                th = sb.tile([128, fd], BF, tag="th", name=f"th{li + 1}_{t}")
                S.activation(th[:, :], zh[:, :], AF.Tanh,
                             bias=bias[:, li + 1:li + 2])
                q = sb.tile([128, fd], BF, tag="q", name=f"q{li + 1}_{t}")
                V.tensor_tensor(q[:, :], th[:, :], th[:, :], ALU.mult)
                tso = sb.tile([128, 3 * fd], BF, tag="tso", name=f"tso{li}_{t}")
                qb = q[:, :].unsqueeze(1).broadcast_to([128, 3, fd])
                V.scalar_tensor_tensor(
                    tso[:, :].rearrange("p (r f) -> p r f", r=3), qb, 1.0,
                    zz[:, :].rearrange("p (r f) -> p r f", r=3),
                    ALU.subtract, ALU.mult)
                zx2 = sb.tile([128, 2 * fd], BF, tag="zx2", name=f"zx2{li}_{t}")
                S.activation(zx2[:, :], zz[:, 0:2 * fd], AF.Square, scale=SQRT2)
                zx2s = sb.tile([128, fd], BF, tag="zx2s", name=f"zx2s{li}_{t}")
                G.tensor_tensor(zx2s[:, :], zx2[:, 0:fd], zx2[:, fd:2 * fd],
                                ALU.add)
                thgm = sb.tile([128, fd], BF, tag="thgm", name=f"thgm{li}_{t}")
                V.scalar_tensor_tensor(thgm[:, :], q[:, :], 1.0, th[:, :],
                                       ALU.subtract, ALU.mult)
                pp = sb.tile([128, fd], BF, tag="pp", name=f"pp{li}_{t}")
                G.tensor_tensor(pp[:, :], thgm[:, :], zx2s[:, :], ALU.mult)
            return th, tso, pp

        def emit_tail(t, th, tso, pp):
            if not DBG_L4:
                return
            off = t * FD
            fd = min(FD, NPC - off)
            nchunks = (fd + 127) // 128

            # ---- L4: stacked output matmuls into o14 [NS, fd] ----
            o14 = pa.tile([NS, fd], F32, tag="pa", name=f"o14_{t}")
            T.matmul(o14[:, :], w4[:, 0:NS], th[:, :], start=True, stop=False)
            T.matmul(o14[:, :], w4[:, NS:2 * NS], tso[:, 0:fd],
                     start=False, stop=False)
            T.matmul(o14[:, :], w4[:, 2 * NS:3 * NS], tso[:, fd:2 * fd],
                     start=False, stop=False)
            T.matmul(o14[:, :], w4[:, 3 * NS:4 * NS], tso[:, 2 * fd:3 * fd],
                     start=False, stop=False)
            T.matmul(o14[:, :], w4[:, 4 * NS:5 * NS], pp[:, :],
                     start=False, stop=True)

            o14sb = sb.tile([NS, fd], F32, tag="o14sb", name=f"o14sb_{t}")
            S.activation(o14sb[:, :], o14[:, :], AF.Identity, bias=b4s[:, 0:1])

            # ---- transpose to points-on-partitions ----
            qt = pa.tile([128, NS * nchunks], F32, tag="pa", name=f"qt_{t}")
            for ci in range(nchunks):
                w = min(128, fd - ci * 128)
                T.transpose(qt[0:w, ci * NS:(ci + 1) * NS],
                            o14sb[:, ci * 128:ci * 128 + w],
                            ident[:, :])
            gw = min(128, fd - (nchunks - 1) * 128)
            qoff = t * NS * 4
            if gw == 128:
                S.copy(qall[:, qoff:qoff + NS * nchunks], qt[:, :])
            else:
                if nchunks > 1:
                    S.copy(qall[:, qoff:qoff + NS * (nchunks - 1)],
                           qt[:, 0:NS * (nchunks - 1)])
                S.copy(
                    qall[0:gw, qoff + NS * (nchunks - 1):qoff + NS * nchunks],
                    qt[0:gw, NS * (nchunks - 1):NS * nchunks])

        prev = None
        for t in range(DBG_NT):
            cur = emit_head(t)
            if prev is not None:
                emit_tail(t - 1, *prev)
            prev = cur
        emit_tail(DBG_NT - 1, *prev)

        # ---- pointwise loss phase on [128, NCH] views ----
        def _pointwise():
            def qv(j):
                return qall[:, j:NS * NCH:NS]

            _ctr = [0]

            def new():
                _ctr[0] += 1
                return scr.tile([128, NCH], F32, tag="scr", name=f"scr{_ctr[0]}")

            def tt(a, b, op, eng=V):
                o = new()
                eng.tensor_tensor(o[:, :], a, b, op)
                return o

            A = tt(qv(4), qv(8), ALU.add, G)             # ux+uy
            B = tt(qv(5), qv(9), ALU.add, V)             # vx+vy
            uv1 = tt(qv(1), A[:, :], ALU.mult, G)        # v*(ux+uy)
            uv2 = tt(qv(0), B[:, :], ALU.mult, V)        # u*(vx+vy)
            uvxy = tt(uv1[:, :], uv2[:, :], ALU.add, V)

            def stt_nu(zz, eng):
                o = new()
                eng.scalar_tensor_tensor(o[:, :], qv(3), NU, zz, ALU.add,
                                         ALU.mult)
                return o

            t1 = stt_nu(qv(12), V)               # (nut+NU)*Lu
            t3 = tt(qv(7), qv(4), ALU.mult, V)   # nux*ux
            t4 = tt(qv(11), qv(8), ALU.mult, G)  # nuy*uy
            a1 = tt(uvxy[:, :], qv(6), ALU.add, V)
            a3 = tt(t3[:, :], t4[:, :], ALU.add, G)
            a4 = tt(t1[:, :], a3[:, :], ALU.add, V)
            f_u = tt(a1[:, :], a4[:, :], ALU.subtract, V)

            t5 = stt_nu(qv(13), V)               # (nut+NU)*Lv
            t7 = tt(qv(7), qv(5), ALU.mult, V)   # nux*vx
            t8 = tt(qv(11), qv(9), ALU.mult, G)  # nuy*vy
            b1 = tt(uvxy[:, :], qv(10), ALU.add, V)
            b3 = tt(t7[:, :], t8[:, :], ALU.add, G)
            b4 = tt(t5[:, :], b3[:, :], ALU.add, V)
            f_v = tt(b1[:, :], b4[:, :], ALU.subtract, V)

            ic = tt(qv(4), qv(9), ALU.add, G)

            t1b = new()
            V.tensor_scalar(t1b[:, :], qv(0), cnb[:, 0:1], cnb[:, 1:2],
                            ALU.mult, ALU.add)
            xnb = new()
            V.tensor_scalar(xnb[:, :], tins[:, 0:NCH], cnb[:, 2:3], cnb[:, 3:4],
                            ALU.mult, ALU.add)
            t2b = new()
            V.tensor_scalar(t2b[:, :], qv(1), cnb[:, 4:5], cnb[:, 5:6],
                            ALU.mult, ALU.add)
            ynb = new()
            V.tensor_scalar(ynb[:, :], tins[:, NCH:2 * NCH], cnb[:, 6:7],
                            cnb[:, 7:8], ALU.mult, ALU.add)
            m1 = tt(t1b[:, :], xnb[:, :], ALU.mult, V)
            m2 = tt(t2b[:, :], ynb[:, :], ALU.mult, G)
            bc0 = tt(m1[:, :], m2[:, :], ALU.add, V)
            bc = tt(bc0[:, :], mask[:, :], ALU.mult, V)

            du = tt(tins[:, 2 * NCH:3 * NCH], qv(0), ALU.subtract, V)
            dv = tt(tins[:, 3 * NCH:4 * NCH], qv(1), ALU.subtract, G)
            dp = tt(tins[:, 4 * NCH:5 * NCH], qv(2), ALU.subtract, V)
            dnut = tt(tins[:, 5 * NCH:6 * NCH], qv(3), ALU.subtract, G)

            for k, val in enumerate([f_u, f_v, bc, ic, du, dv, dp, dnut]):
                o = new()
                S.activation(o[:, :], val[:, :], AF.Square,
                             accum_out=souts[:, k:k + 1])

            nc.sync.dma_start(out=d_out[:, :], in_=souts[:, :])

        if DBG_PW:
            _pointwise()
        else:
            nc.sync.dma_start(out=d_out[:, :], in_=qall[:, 0:8])

    nc.compile()
    return nc


def _prep_core(inputs, c):
    s = slice(c * NPC, (c + 1) * NPC)
    f32 = np.float32
    col = lambda k: np.asarray(inputs[k], f32)[s, 0]
    feat = np.ascontiguousarray(np.stack([
        col("x"), col("y"), col("x_normal"), col("y_normal"), col("sdf"),
        col("gamma_1"), col("gamma_2"), col("gamma_3")]))
    W = [np.asarray(inputs[f"W{i}"], f32) for i in range(5)]
    b = [np.asarray(inputs[f"b{i}"], f32) for i in range(5)]
    cn = np.asarray(inputs["coef_norm"], f32)

    w0x, w0y = W[0][0, :], W[0][1, :]
    w0s = 2.0 * (w0x * w0x + w0y * w0y)
    wh = np.concatenate([W[1], W[2], W[3]], axis=1)
    wx = np.concatenate([W[1] * w0x[:, None], W[1] * w0y[:, None],
                         W[1] * w0s[:, None]], axis=1)
    w4sp = np.zeros((128, 5 * NS), f32)
    w4sp[:, 0:4] = W[4]                       # slot th -> rows 0:4
    w4sp[:, NS + 4:NS + 8] = W[4]             # slot t_x -> rows 4:8
    w4sp[:, 2 * NS + 8:2 * NS + 12] = W[4]    # slot t_y -> rows 8:12
    w4sp[:, 3 * NS + 12:3 * NS + 14] = -W[4][:, 0:2]  # slot s -> rows 12:14
    w4sp[:, 4 * NS + 12:4 * NS + 14] = W[4][:, 0:2]   # slot pp -> rows 12:14

    bias = np.stack([b[0], b[1], b[2], b[3]], axis=1)
    b4s = np.concatenate([b[4], np.zeros(NS - 4, f32)])[:, None]
    cnv = np.array([cn[3, 0] + EPS, cn[2, 0], cn[1, 5] + EPS, cn[0, 5],
                    cn[3, 1] + EPS, cn[2, 1], cn[1, 6] + EPS, cn[0, 6]], f32)
    cnb = np.broadcast_to(cnv, (128, 8)).copy()

    def tcol(k):
        a = np.zeros(NCH * 128, f32)
        a[:NPC] = col(k)
        return a.reshape(NCH, 128).T  # [128, NCH]

    tins = np.ascontiguousarray(np.concatenate(
        [tcol("x_normal"), tcol("y_normal"), tcol("u0"), tcol("v0"),
         tcol("p0"), tcol("nut0")], axis=1))
    m = np.zeros(NCH * 128, f32)
    m[:NPC] = 1.0
    mask = np.ascontiguousarray(m.reshape(NCH, 128).T)

    bf = lambda a: np.ascontiguousarray(a.astype(BF_NP))
    return {
        "featb": bf(feat), "w0b": bf(W[0]), "wh": bf(wh), "wx": bf(wx),
        "w2n": bf(-W[2]),
        "w4sp": bf(w4sp),
        "bias": np.ascontiguousarray(bias), "b4s": b4s,
        "cnb": cnb, "tins": tins, "mask": mask,
        "ident": np.eye(NS, dtype=f32),
    }


def _get_nc():
    if "nc" not in _CACHE:
        _CACHE["nc"] = _build()
    return _CACHE["nc"]


def run_device(inputs, **kw):
    nc = _get_nc()
    in_maps = [_prep_core(inputs, c) for c in range(NCORES)]
    res = run_bass_kernel_spmd(nc, in_maps, core_ids=list(range(NCORES)), **kw)
    return res


def _combine(results):
    S = np.stack([r["sout"] for r in results]).astype(np.float64)  # [8,128,8]
    m = S.sum(axis=(0, 1)) / NPTS
    rans, bcl, icl = m[0] + m[1], m[2], m[3]
    ul, vl, pl, nl = m[4], m[5], m[6], m[7]
    inlet = ul + vl + pl + nl
    total = rans + bcl + inlet + icl
    return np.array([total, rans, bcl, inlet, icl, ul, vl, pl, nl],
                    dtype=np.float32)


def kernel(**inputs):
    res = run_device(inputs)
    return _combine(res.results)


# revision 9
# speedup vs baseline: 1.1913x; 1.1386x over previous
"""PINN loss kernel for trn2 (8 NeuronCores, data-parallel over points).

v2: forward-mode AD with a single Laplacian second-derivative stream
(s = s_xx + s_yy; the losses only need u_xx+u_yy / v_xx+v_yy), bf16
streams + matmuls, layer-1 tangent scales folded into pre-scaled weight
copies (host-side), engine-balanced elementwise work.

Layout: features on partitions, points on the free dim; per-core shard of
6250 points processed in 13 tiles of <=512 points. Per-core partial sums
[128, 8] are combined on host (the unshard step).
"""

import os
import sys

for _p in ("/opt/trn_rl_repo", "/root/.axon_site/_ro/trn_rl_repo"):
    if os.path.isdir(_p) and _p not in sys.path:
        sys.path.insert(0, _p)

import numpy as np
import ml_dtypes
from contextlib import ExitStack

from concourse import bass, bacc, tile, mybir
from concourse.bass_utils import run_bass_kernel_spmd

NCORES = 8
NPTS = 50000
NPC = NPTS // NCORES          # 6250 points per core
FD = 512                      # points per tile
NT = (NPC + FD - 1) // FD     # 13 tiles (12 full + one of 106)
NCH = (NPC + 127) // 128      # 49 transpose chunks per core
NS = 14                       # output slots per point
NU = 1.56e-05
EPS = 1e-08
SQRT2 = 1.4142135623730951

F32 = mybir.dt.float32
BF = mybir.dt.float16
AF = mybir.ActivationFunctionType
ALU = mybir.AluOpType
BF_NP = np.float16

DBG_NT = int(os.environ.get("PINN_NT", NT))          # tiles to emit (debug)
DBG_L4 = os.environ.get("PINN_SKIP_L4", "") == ""    # emit L4+transpose
DBG_PW = os.environ.get("PINN_SKIP_PW", "") == ""    # emit pointwise phase

_CACHE = {}


def _build():
    nc = bacc.Bacc("TRN2", target_bir_lowering=False, debug=False)

    # ---- DRAM I/O ----
    d_feat = nc.dram_tensor("featb", [8, NPC], BF, kind="ExternalInput")
    d_w0 = nc.dram_tensor("w0b", [8, 128], BF, kind="ExternalInput")
    d_wh = nc.dram_tensor("wh", [128, 128 * 3], BF, kind="ExternalInput")
    d_wx = nc.dram_tensor("wx", [128, 128 * 3], BF, kind="ExternalInput")
    d_w2n = nc.dram_tensor("w2n", [128, 128], BF, kind="ExternalInput")
    d_w4 = nc.dram_tensor("w4sp", [128, 5 * NS], BF, kind="ExternalInput")
    d_bias = nc.dram_tensor("bias", [128, 4], F32, kind="ExternalInput")
    d_b4s = nc.dram_tensor("b4s", [NS, 1], F32, kind="ExternalInput")
    d_cnb = nc.dram_tensor("cnb", [128, 8], F32, kind="ExternalInput")
    d_tins = nc.dram_tensor("tins", [128, 6 * NCH], F32, kind="ExternalInput")
    d_mask = nc.dram_tensor("mask", [128, NCH], F32, kind="ExternalInput")
    d_id = nc.dram_tensor("ident", [NS, NS], F32, kind="ExternalInput")
    d_out = nc.dram_tensor("sout", [128, 8], F32, kind="ExternalOutput")

    with tile.TileContext(nc) as tc, ExitStack() as ctx:
        wp = ctx.enter_context(tc.tile_pool(name="wp", bufs=1))
        sb = ctx.enter_context(tc.tile_pool(name="sb", bufs=int(os.environ.get("PINN_SBUFS", "4"))))
        scr = ctx.enter_context(tc.tile_pool(name="scr", bufs=26))
        pa = ctx.enter_context(tc.tile_pool(name="pa", bufs=2, space="PSUM"))
        pb = ctx.enter_context(tc.tile_pool(name="pb", bufs=2, space="PSUM"))

        # ---- persistent sbuf tensors ----
        feat = wp.tile([8, NPC], BF, tag="feat")
        w0 = wp.tile([8, 128], BF, tag="w0")
        wh = wp.tile([128, 128 * 3], BF, tag="wh")
        wx = wp.tile([128, 128 * 3], BF, tag="wx")
        w2n = wp.tile([128, 128], BF, tag="w2n")
        w4 = wp.tile([128, 5 * NS], BF, tag="w4")
        bias = wp.tile([128, 4], F32, tag="bias")
        b4s = wp.tile([NS, 1], F32, tag="b4s")
        cnb = wp.tile([128, 8], F32, tag="cnb")
        tins = wp.tile([128, 6 * NCH], F32, tag="tins")
        mask = wp.tile([128, NCH], F32, tag="mask")
        ident = wp.tile([NS, NS], F32, tag="ident")
        qall = wp.tile([128, NS * NCH], F32, tag="qall")
        souts = wp.tile([128, 8], F32, tag="souts")

        warm = wp.tile([1, 8], F32, tag="warm")
        nc.gpsimd.memset(warm[:, :], 0.25)
        nc.scalar.activation(warm[:, :], warm[:, :], AF.Tanh)
        nc.scalar.activation(warm[:, :], warm[:, :], AF.Square)
        dma = nc.sync.dma_start
        dma(out=feat[:, :], in_=d_feat[:, :])
        dma(out=w0[:, :], in_=d_w0[:, :])
        dma(out=wh[:, :], in_=d_wh[:, :])
        dma(out=wx[:, :], in_=d_wx[:, :])
        dma(out=w2n[:, :], in_=d_w2n[:, :])
        dma(out=w4[:, :], in_=d_w4[:, :])
        dma(out=bias[:, :], in_=d_bias[:, :])
        dma(out=b4s[:, :], in_=d_b4s[:, :])
        dma(out=cnb[:, :], in_=d_cnb[:, :])
        dma(out=tins[:, :], in_=d_tins[:, :])
        dma(out=mask[:, :], in_=d_mask[:, :])
        dma(out=ident[:, :], in_=d_id[:, :])
        nc.gpsimd.memset(qall[:, :], 0.0)

        V, S, G, T = nc.vector, nc.scalar, nc.gpsimd, nc.tensor
        SGN = [-1.0, 1.0, -1.0]  # sign of stored pp term per hidden iter

        def emit_el1(t):
            off = t * FD
            fd = min(FD, NPC - off)

            # ---- L0 matmul + EL1 ----
            zh = pa.tile([128, fd], F32, tag="pa", name=f"zh0_{t}")
            T.matmul(zh[:, :], w0[:, :], feat[:, off:off + fd],
                     start=True, stop=True)
            th = sb.tile([128, fd], BF, tag="th", name=f"th0_{t}")
            S.activation(th[:, :], zh[:, :], AF.Tanh, bias=bias[:, 0:1])
            q = sb.tile([128, fd], BF, tag="q", name=f"q0_{t}")
            V.tensor_tensor(q[:, :], th[:, :], th[:, :], ALU.mult)
            qm1 = sb.tile([128, fd], BF, tag="qm1", name=f"qm1_{t}")
            V.tensor_scalar_add(qm1[:, :], q[:, :], -1.0)
            thgm = sb.tile([128, fd], BF, tag="thgm", name=f"thgm0_{t}")
            G.tensor_tensor(thgm[:, :], qm1[:, :], th[:, :], ALU.mult)
            return {"th": th, "qm1": qm1, "thgm": thgm, "tso": None, "pp": None}

        # ---- hidden iteration li (weights W1,W2,W3) ----
        def emit_hidden(t, li, st):
            off = t * FD
            fd = min(FD, NPC - off)
            th, qm1, thgm = st["th"], st["qm1"], st["thgm"]
            tso, pp = st["tso"], st["pp"]
            if True:
                W = wh[:, li * 128:(li + 1) * 128]
                zh = pa.tile([128, fd], F32, tag="pa", name=f"zh{li}_{t}")
                zz = pb.tile([128, 3 * fd], F32, tag="pb", name=f"zz{li}_{t}")
                if li == 0:
                    T.matmul(zh[:, :], W, th[:, :], start=True, stop=True)
                    T.matmul(zz[:, 0:fd], wx[:, 0:128], qm1[:, :],
                             start=True, stop=True)
                    T.matmul(zz[:, fd:2 * fd], wx[:, 128:256], qm1[:, :],
                             start=True, stop=True)
                    T.matmul(zz[:, 2 * fd:3 * fd], wx[:, 256:384], thgm[:, :],
                             start=True, stop=True)
                else:
                    T.matmul(zh[:, :], W, th[:, :], start=True, stop=True)
                    T.matmul(zz[:, 0:fd], W, tso[:, 0:fd], start=True, stop=True)
                    T.matmul(zz[:, fd:2 * fd], W, tso[:, fd:2 * fd],
                             start=True, stop=True)
                    T.matmul(zz[:, 2 * fd:3 * fd], W, tso[:, 2 * fd:3 * fd],
                             start=True, stop=False)
                    Wpp = w2n[:, :] if li == 1 else W
                    T.matmul(zz[:, 2 * fd:3 * fd], Wpp, pp[:, :],
                             start=False, stop=True)

                th = sb.tile([128, fd], BF, tag="th", name=f"th{li + 1}_{t}")
                S.activation(th[:, :], zh[:, :], AF.Tanh,
                             bias=bias[:, li + 1:li + 2])
                q = sb.tile([128, fd], BF, tag="q", name=f"q{li + 1}_{t}")
                V.tensor_tensor(q[:, :], th[:, :], th[:, :], ALU.mult)
                qm1 = sb.tile([128, fd], BF, tag="qm1", name=f"qm1{li}_{t}")
                V.tensor_scalar_add(qm1[:, :], q[:, :], -1.0)
                tso = sb.tile([128, 3 * fd], BF, tag="tso", name=f"tso{li}_{t}")
                qb = q[:, :].unsqueeze(1).broadcast_to([128, 3, fd])
                V.scalar_tensor_tensor(
                    tso[:, :].rearrange("p (r f) -> p r f", r=3), qb, 1.0,
                    zz[:, :].rearrange("p (r f) -> p r f", r=3),
                    ALU.subtract, ALU.mult)
                zx2 = sb.tile([128, 2 * fd], BF, tag="zx2", name=f"zx2{li}_{t}")
                S.activation(zx2[:, :], zz[:, 0:2 * fd], AF.Square, scale=SQRT2)
                zx2s = sb.tile([128, fd], BF, tag="zx2s", name=f"zx2s{li}_{t}")
                G.tensor_tensor(zx2s[:, :], zx2[:, 0:fd], zx2[:, fd:2 * fd],
                                ALU.add)
                thgm = sb.tile([128, fd], BF, tag="thgm", name=f"thgm{li}_{t}")
                G.tensor_tensor(thgm[:, :], qm1[:, :], th[:, :], ALU.mult)
                pp = sb.tile([128, fd], BF, tag="pp", name=f"pp{li}_{t}")
                V.tensor_tensor(pp[:, :], thgm[:, :], zx2s[:, :], ALU.mult)
            st["th"], st["tso"], st["pp"] = th, tso, pp

        def emit_tail(t, st):
            th, tso, pp = st["th"], st["tso"], st["pp"]
            if not DBG_L4:
                return
            off = t * FD
            fd = min(FD, NPC - off)
            nchunks = (fd + 127) // 128

            # ---- L4: stacked output matmuls into o14 [NS, fd] ----
            o14 = pa.tile([NS, fd], F32, tag="pa", name=f"o14_{t}")
            T.matmul(o14[:, :], w4[:, 0:NS], th[:, :], start=True, stop=False)
            T.matmul(o14[:, :], w4[:, NS:2 * NS], tso[:, 0:fd],
                     start=False, stop=False)
            T.matmul(o14[:, :], w4[:, 2 * NS:3 * NS], tso[:, fd:2 * fd],
                     start=False, stop=False)
            T.matmul(o14[:, :], w4[:, 3 * NS:4 * NS], tso[:, 2 * fd:3 * fd],
                     start=False, stop=False)
            T.matmul(o14[:, :], w4[:, 4 * NS:5 * NS], pp[:, :],
                     start=False, stop=True)

            o14sb = sb.tile([NS, fd], F32, tag="o14sb", name=f"o14sb_{t}")
            S.activation(o14sb[:, :], o14[:, :], AF.Identity, bias=b4s[:, 0:1])

            # ---- transpose to points-on-partitions ----
            qt = pa.tile([128, NS * nchunks], F32, tag="pa", name=f"qt_{t}")
            for ci in range(nchunks):
                w = min(128, fd - ci * 128)
                T.transpose(qt[0:w, ci * NS:(ci + 1) * NS],
                            o14sb[:, ci * 128:ci * 128 + w],
                            ident[:, :])
            gw = min(128, fd - (nchunks - 1) * 128)
            qoff = t * NS * 4
            if gw == 128:
                S.copy(qall[:, qoff:qoff + NS * nchunks], qt[:, :])
            else:
                if nchunks > 1:
                    S.copy(qall[:, qoff:qoff + NS * (nchunks - 1)],
                           qt[:, 0:NS * (nchunks - 1)])
                S.copy(
                    qall[0:gw, qoff + NS * (nchunks - 1):qoff + NS * nchunks],
                    qt[0:gw, NS * (nchunks - 1):NS * nchunks])

        for p in range(0, DBG_NT, 2):
            ts = [t for t in (p, p + 1) if t < DBG_NT]
            sts = {}
            for t in ts:
                sts[t] = emit_el1(t)
            for li in range(3):
                for t in ts:
                    emit_hidden(t, li, sts[t])
            for t in ts:
                emit_tail(t, sts[t])

        # ---- pointwise loss phase on [128, NCH] views ----
        def _pointwise():
            def qv(j):
                return qall[:, j:NS * NCH:NS]

            _ctr = [0]

            def new():
                _ctr[0] += 1
                return scr.tile([128, NCH], F32, tag="scr", name=f"scr{_ctr[0]}")

            def tt(a, b, op, eng=V):
                o = new()
                eng.tensor_tensor(o[:, :], a, b, op)
                return o

            A = tt(qv(4), qv(8), ALU.add, G)             # ux+uy
            B = tt(qv(5), qv(9), ALU.add, V)             # vx+vy
            uv1 = tt(qv(1), A[:, :], ALU.mult, G)        # v*(ux+uy)
            uv2 = tt(qv(0), B[:, :], ALU.mult, V)        # u*(vx+vy)
            uvxy = tt(uv1[:, :], uv2[:, :], ALU.add, V)

            def stt_nu(zz, eng):
                o = new()
                eng.scalar_tensor_tensor(o[:, :], qv(3), NU, zz, ALU.add,
                                         ALU.mult)
                return o

            t1 = stt_nu(qv(12), V)               # (nut+NU)*Lu
            t3 = tt(qv(7), qv(4), ALU.mult, V)   # nux*ux
            t4 = tt(qv(11), qv(8), ALU.mult, G)  # nuy*uy
            a1 = tt(uvxy[:, :], qv(6), ALU.add, V)
            a3 = tt(t3[:, :], t4[:, :], ALU.add, G)
            a4 = tt(t1[:, :], a3[:, :], ALU.add, V)
            f_u = tt(a1[:, :], a4[:, :], ALU.subtract, V)

            t5 = stt_nu(qv(13), V)               # (nut+NU)*Lv
            t7 = tt(qv(7), qv(5), ALU.mult, V)   # nux*vx
            t8 = tt(qv(11), qv(9), ALU.mult, G)  # nuy*vy
            b1 = tt(uvxy[:, :], qv(10), ALU.add, V)
            b3 = tt(t7[:, :], t8[:, :], ALU.add, G)
            b4 = tt(t5[:, :], b3[:, :], ALU.add, V)
            f_v = tt(b1[:, :], b4[:, :], ALU.subtract, V)

            ic = tt(qv(4), qv(9), ALU.add, G)

            t1b = new()
            V.tensor_scalar(t1b[:, :], qv(0), cnb[:, 0:1], cnb[:, 1:2],
                            ALU.mult, ALU.add)
            xnb = new()
            V.tensor_scalar(xnb[:, :], tins[:, 0:NCH], cnb[:, 2:3], cnb[:, 3:4],
                            ALU.mult, ALU.add)
            t2b = new()
            V.tensor_scalar(t2b[:, :], qv(1), cnb[:, 4:5], cnb[:, 5:6],
                            ALU.mult, ALU.add)
            ynb = new()
            V.tensor_scalar(ynb[:, :], tins[:, NCH:2 * NCH], cnb[:, 6:7],
                            cnb[:, 7:8], ALU.mult, ALU.add)
            m1 = tt(t1b[:, :], xnb[:, :], ALU.mult, V)
            m2 = tt(t2b[:, :], ynb[:, :], ALU.mult, G)
            bc0 = tt(m1[:, :], m2[:, :], ALU.add, V)
            bc = tt(bc0[:, :], mask[:, :], ALU.mult, V)

            du = tt(tins[:, 2 * NCH:3 * NCH], qv(0), ALU.subtract, V)
            dv = tt(tins[:, 3 * NCH:4 * NCH], qv(1), ALU.subtract, G)
            dp = tt(tins[:, 4 * NCH:5 * NCH], qv(2), ALU.subtract, V)
            dnut = tt(tins[:, 5 * NCH:6 * NCH], qv(3), ALU.subtract, G)

            for k, val in enumerate([f_u, f_v, bc, ic, du, dv, dp, dnut]):
                o = new()
                S.activation(o[:, :], val[:, :], AF.Square,
                             accum_out=souts[:, k:k + 1])

            nc.sync.dma_start(out=d_out[:, :], in_=souts[:, :])

        if DBG_PW:
            _pointwise()
        else:
            nc.sync.dma_start(out=d_out[:, :], in_=qall[:, 0:8])

    nc.compile()
    return nc


def _prep_core(inputs, c):
    s = slice(c * NPC, (c + 1) * NPC)
    f32 = np.float32
    col = lambda k: np.asarray(inputs[k], f32)[s, 0]
    feat = np.ascontiguousarray(np.stack([
        col("x"), col("y"), col("x_normal"), col("y_normal"), col("sdf"),
        col("gamma_1"), col("gamma_2"), col("gamma_3")]))
    W = [np.asarray(inputs[f"W{i}"], f32) for i in range(5)]
    b = [np.asarray(inputs[f"b{i}"], f32) for i in range(5)]
    cn = np.asarray(inputs["coef_norm"], f32)

    w0x, w0y = W[0][0, :], W[0][1, :]
    w0s = 2.0 * (w0x * w0x + w0y * w0y)
    wh = np.concatenate([W[1], W[2], W[3]], axis=1)
    wx = np.concatenate([W[1] * w0x[:, None], W[1] * w0y[:, None],
                         W[1] * w0s[:, None]], axis=1)
    w4sp = np.zeros((128, 5 * NS), f32)
    w4sp[:, 0:4] = W[4]                       # slot th -> rows 0:4
    w4sp[:, NS + 4:NS + 8] = W[4]             # slot t_x -> rows 4:8
    w4sp[:, 2 * NS + 8:2 * NS + 12] = W[4]    # slot t_y -> rows 8:12
    w4sp[:, 3 * NS + 12:3 * NS + 14] = -W[4][:, 0:2]  # slot s -> rows 12:14
    w4sp[:, 4 * NS + 12:4 * NS + 14] = W[4][:, 0:2]   # slot pp -> rows 12:14

    bias = np.stack([b[0], b[1], b[2], b[3]], axis=1)
    b4s = np.concatenate([b[4], np.zeros(NS - 4, f32)])[:, None]
    cnv = np.array([cn[3, 0] + EPS, cn[2, 0], cn[1, 5] + EPS, cn[0, 5],
                    cn[3, 1] + EPS, cn[2, 1], cn[1, 6] + EPS, cn[0, 6]], f32)
    cnb = np.broadcast_to(cnv, (128, 8)).copy()

    def tcol(k):
        a = np.zeros(NCH * 128, f32)
        a[:NPC] = col(k)
        return a.reshape(NCH, 128).T  # [128, NCH]

    tins = np.ascontiguousarray(np.concatenate(
        [tcol("x_normal"), tcol("y_normal"), tcol("u0"), tcol("v0"),
         tcol("p0"), tcol("nut0")], axis=1))
    m = np.zeros(NCH * 128, f32)
    m[:NPC] = 1.0
    mask = np.ascontiguousarray(m.reshape(NCH, 128).T)

    bf = lambda a: np.ascontiguousarray(a.astype(BF_NP))
    return {
        "featb": bf(feat), "w0b": bf(W[0]), "wh": bf(wh), "wx": bf(wx),
        "w2n": bf(-W[2]),
        "w4sp": bf(w4sp),
        "bias": np.ascontiguousarray(bias), "b4s": b4s,
        "cnb": cnb, "tins": tins, "mask": mask,
        "ident": np.eye(NS, dtype=f32),
    }


def _get_nc():
    if "nc" not in _CACHE:
        _CACHE["nc"] = _build()
    return _CACHE["nc"]


def run_device(inputs, **kw):
    nc = _get_nc()
    in_maps = [_prep_core(inputs, c) for c in range(NCORES)]
    res = run_bass_kernel_spmd(nc, in_maps, core_ids=list(range(NCORES)), **kw)
    return res


def _combine(results):
    S = np.stack([r["sout"] for r in results]).astype(np.float64)  # [8,128,8]
    m = S.sum(axis=(0, 1)) / NPTS
    rans, bcl, icl = m[0] + m[1], m[2], m[3]
    ul, vl, pl, nl = m[4], m[5], m[6], m[7]
    inlet = ul + vl + pl + nl
    total = rans + bcl + inlet + icl
    return np.array([total, rans, bcl, inlet, icl, ul, vl, pl, nl],
                    dtype=np.float32)


def kernel(**inputs):
    res = run_device(inputs)
    return _combine(res.results)
